# revision 4
# baseline (speedup 1.0000x reference)
"""Trainium2 Bass kernel for nn_Attention (B=4, N=2048, D=1024, H=16, Hd=64).

Sharding: 8 cores = 4 batches x 2 head-groups; core c: batch c//2, heads
[(c%2)*8, (c%2)*8+8). Host sums the two partial projections per batch + bias.

v2 design (vs v1): keeps ScalarE (the 257us exp floor) saturated and pushes
TensorE below it via PE array tiling:
  - scores run as 4-way-concurrent 64x64 array tiles (2 heads x keys-lo/hi),
    ~216ns per [128 keys x 2x512 q] group (measured), using a 6-slice PSUM
    rotation [128,3072] so consecutive steps' chunks never WAR-block.
  - exp ops are [128,1024] PSUM->SBUF (1005ns measured) and always have
    inputs ready >=2 ops ahead (slice reuse distance 1.5 kt).
  - qkv/proj dense chains + pv (attn@V) all run as (128,64) column-paired
    MMs so per-step the PE sees only two tiling-mode switches.
  - softmax denominators: eacc adds on DVE (+ every 4th on GpSimd),
    ones-matmul partition reduce, reciprocal + gpsimd broadcast, with the
    normalize fused into the PSUM->SBUF drain (tensor_mul from PSUM).
  - U^T accumulates in a single PSUM bank [128,512] per (half, q-slice);
    the two q-slice passes per half are pipelined across half boundaries.
"""

import os
import sys
import types

import numpy as np

for _p in ("/opt/trn_rl_repo", "/root/.axon_site/_ro/trn_rl_repo"):
    if _p not in sys.path and os.path.isdir(_p):
        sys.path.append(_p)

import ml_dtypes  # noqa: E402

BF16 = ml_dtypes.bfloat16


def _install_ntff_shim():
    if "antenv.axon_hooks" in sys.modules:
        return
    mod = types.ModuleType("antenv.axon_hooks")
    mod._hook = None
    mod.set_axon_ntff_profile_hook = lambda h: setattr(mod, "_hook", h)
    mod.get_axon_ntff_profile_hook = lambda: mod._hook
    sys.modules["antenv.axon_hooks"] = mod
    try:
        import antenv

        antenv.axon_hooks = mod
    except ImportError:
        pass
    try:
        from trn_agent_boot.trn_boot import _ntff_profile_via_ctypes

        hook = _ntff_profile_via_ctypes("/opt/axon/libaxon_pjrt.so")
        if hook is not None:
            mod.set_axon_ntff_profile_hook(hook)
    except Exception:
        pass


_install_ntff_shim()

import concourse.bacc as bacc  # noqa: E402
import concourse.tile as tile  # noqa: E402
from concourse import mybir  # noqa: E402
import concourse.bass_utils as bass_utils  # noqa: E402

bass_utils.upload_artifacts = lambda tmpdir: tmpdir

F32 = mybir.dt.float32
BF = mybir.dt.bfloat16
EXP = mybir.ActivationFunctionType.Exp

N_CORES = 8
NT = 2048
D = 1024
HD = 64
SCALE = HD**-0.5

HALVES = [(0, 0), (1, 0), (2, 0), (3, 0), (0, 1), (1, 1), (2, 1), (3, 1)]


def _body(tc: "tile.TileContext", ctx, y, xT, wqk, wv, wp):
    nc = tc.nc

    wpool = ctx.enter_context(tc.tile_pool(name="wpool", bufs=1))
    e0pool = ctx.enter_context(tc.tile_pool(name="e0pool", bufs=8))
    e1pool = ctx.enter_context(tc.tile_pool(name="e1pool", bufs=17))
    eapool = ctx.enter_context(tc.tile_pool(name="eapool", bufs=2))
    spool = ctx.enter_context(tc.tile_pool(name="spool", bufs=2))
    recpool = ctx.enter_context(tc.tile_pool(name="recpool", bufs=4))
    opool = ctx.enter_context(tc.tile_pool(name="opool", bufs=2))
    psS = ctx.enter_context(tc.tile_pool(name="psS", bufs=1, space="PSUM"))
    psU = ctx.enter_context(tc.tile_pool(name="psU", bufs=1, space="PSUM"))
    psW = ctx.enter_context(tc.tile_pool(name="psW", bufs=1, space="PSUM"))

    # ---- persistent SBUF + ordered input DMAs ----------------------------
    xT_sb = [wpool.tile([128, NT], BF, tag=f"xT{i}", name=f"xT{i}") for i in range(8)]
    wqk_sb = [wpool.tile([128, 1024], BF, tag=f"wqk{i}", name=f"wqk{i}") for i in range(8)]
    wv_sb = [wpool.tile([128, 512], BF, tag=f"wv{i}", name=f"wv{i}") for i in range(8)]
    wp_sb = [wpool.tile([128, 1024], BF, tag=f"wp{i}", name=f"wp{i}") for i in range(4)]
    # earliest needs first: x tokens 0:1024, pair-0 qk weights, wv
    for ts in range(2):
        for i in range(8):
            nc.sync.dma_start(out=xT_sb[i][:, ts * 512:(ts + 1) * 512],
                              in_=xT[i * 128:(i + 1) * 128, ts * 512:(ts + 1) * 512])
    for i in range(8):
        nc.sync.dma_start(out=wqk_sb[i][:, 0:256], in_=wqk[i * 128:(i + 1) * 128, 0:256])
    for i in range(8):
        nc.sync.dma_start(out=wv_sb[i], in_=wv[i * 128:(i + 1) * 128, :])
    for ts in range(2, 4):
        for i in range(8):
            nc.sync.dma_start(out=xT_sb[i][:, ts * 512:(ts + 1) * 512],
                              in_=xT[i * 128:(i + 1) * 128, ts * 512:(ts + 1) * 512])
    for p in range(1, 4):
        for i in range(8):
            nc.sync.dma_start(out=wqk_sb[i][:, p * 256:(p + 1) * 256],
                              in_=wqk[i * 128:(i + 1) * 128, p * 256:(p + 1) * 256])
    for i in range(4):
        nc.sync.dma_start(out=wp_sb[i], in_=wp[i * 128:(i + 1) * 128, :])

    # qkT[2p] = Q features of pair p (rows: headA 0:64 | headB 64:128 hd),
    # qkT[2p+1] = K features of pair p.  cols = 2048 tokens.
    qkT = [wpool.tile([128, NT], BF, tag=f"qkT{f}", name=f"qkT{f}") for f in range(8)]
    v_sb = [wpool.tile([128, 512], BF, tag=f"v{t}", name=f"v{t}") for t in range(16)]
    uhat = [wpool.tile([128, NT], BF, tag=f"uh{p}", name=f"uh{p}") for p in range(4)]
    ones64 = wpool.tile([128, 64], BF, tag="ones64", name="ones64")
    nc.vector.memset(ones64, 1.0)

    stbig = psS.tile([128, 3072], F32, tag="stbig", name="stbig")  # 6 banks

    # ---- dense chains (all in (128,64) col-paired mode) ------------------
    def qk_chain(f, ts2):
        scr = psW.tile([128, 512], F32, tag="scr", name=f"qk{f}_{ts2}")
        mv = lambda d: xT_sb[d][:, ts2 * 512:(ts2 + 1) * 512]
        for d in range(8):
            st, sp = (d == 0), (d == 7)
            nc.tensor.matmul(scr[0:64, :], wqk_sb[d][:, f * 128:f * 128 + 64], mv(d), start=st, stop=sp)
            nc.tensor.matmul(scr[64:128, :], wqk_sb[d][:, f * 128 + 64:(f + 1) * 128], mv(d), start=st, stop=sp)
        nc.vector.tensor_copy(out=qkT[f][:, ts2 * 512:(ts2 + 1) * 512], in_=scr)

    def v_chain(t):
        scr = psW.tile([128, 512], F32, tag="scr", name=f"v{t}")
        for d in range(8):
            st, sp = (d == 0), (d == 7)
            nc.tensor.matmul(scr[0:64, :], xT_sb[d][:, t * 128:t * 128 + 64], wv_sb[d], start=st, stop=sp)
            nc.tensor.matmul(scr[64:128, :], xT_sb[d][:, t * 128 + 64:(t + 1) * 128], wv_sb[d], start=st, stop=sp)
        nc.vector.tensor_copy(out=v_sb[t], in_=scr)
        v_emitted[t] = True

    def proj_chain(qt, es):
        scr = psW.tile([128, 512], F32, tag="scr", name=f"pj{qt}_{es}")
        for c in range(4):
            st, sp = (c == 0), (c == 3)
            mv = wp_sb[c][:, es * 512:(es + 1) * 512]
            nc.tensor.matmul(scr[0:64, :], uhat[c][:, qt * 128:qt * 128 + 64], mv, start=st, stop=sp)
            nc.tensor.matmul(scr[64:128, :], uhat[c][:, qt * 128 + 64:(qt + 1) * 128], mv, start=st, stop=sp)
        ot = opool.tile([128, 512], F32, tag="out", name=f"ot{qt}_{es}")
        nc.vector.tensor_copy(out=ot, in_=scr)
        nc.sync.dma_start(out=y[qt * 128:(qt + 1) * 128, es * 512:(es + 1) * 512], in_=ot)

    # ---- attention state --------------------------------------------------
    v_emitted = [False] * 16
    gc = [0]  # global chunk counter -> stbig slice rotation
    eS = {}  # (hidx, s, kt) -> exp'd scores [128, 1024] = [A | B]
    eacc = {}  # (hidx, s) -> accumulated E
    recs = {}  # (hidx, s) -> (recA, recB)
    passes = []  # pv pass FIFO: dicts

    def emit_scores_exps(hidx, p, h, kt):
        qk_q, qk_k = qkT[2 * p], qkT[2 * p + 1]
        sl = [(gc[0] + c) % 6 for c in range(4)]
        gc[0] += 4
        for s in range(2):  # chunk pair (A, B) for q-slice s
            q0 = h * 1024 + s * 512
            for head in range(2):
                r = slice(head * 64, head * 64 + 64)
                ch = stbig[:, sl[2 * s + head] * 512:(sl[2 * s + head] + 1) * 512]
                for lh in range(2):
                    k0 = kt * 128 + lh * 64
                    nc.tensor.matmul(ch[lh * 64:(lh + 1) * 64, :], qk_k[r, k0:k0 + 64],
                                     qk_q[r, q0:q0 + 512], start=True, stop=True)
        for s, pool in ((0, e0pool), (1, e1pool)):
            et = pool.tile([128, 1024], BF, tag=f"e{s}", name=f"e{hidx}_{s}_{kt}")
            a = sl[2 * s]
            nc.scalar.activation(out=et, in_=stbig[:, a * 512:(a + 2) * 512], func=EXP, scale=SCALE)
            eS[(hidx, s, kt)] = et
            if kt == 0:
                ea = eapool.tile([128, 1024], BF, tag=f"ea{s}", name=f"ea{hidx}_{s}")
                eacc[(hidx, s)] = ea
                nc.vector.tensor_copy(out=ea, in_=et)
            elif kt % 4 == 2:
                nc.gpsimd.tensor_add(out=eacc[(hidx, s)], in0=eacc[(hidx, s)], in1=et)
            else:
                nc.vector.tensor_add(out=eacc[(hidx, s)], in0=eacc[(hidx, s)], in1=et)

    def rec_chain(hidx, s):
        ea = eacc[(hidx, s)]
        sp = psW.tile([128, 512], F32, tag="scr", name=f"sums{hidx}_{s}")
        nc.tensor.matmul(sp[0:64, :], ones64, ea[:, 0:512], start=True, stop=True)
        nc.tensor.matmul(sp[64:128, :], ones64, ea[:, 512:1024], start=True, stop=True)
        ss = spool.tile([128, 512], F32, tag="sums", name=f"ss{hidx}_{s}")
        nc.vector.tensor_copy(out=ss, in_=sp)
        out = []
        for hb in (0, 1):
            rsp = spool.tile([128, 4], F32, tag="rsp", name=f"rsp{hidx}_{s}_{hb}")
            row = ss[hb * 64:hb * 64 + 1, :].rearrange("p (a b) -> p a b", a=128)
            nc.gpsimd.dma_start(out=rsp, in_=row)
            rspr = spool.tile([128, 4], F32, tag="rspr", name=f"rspr{hidx}_{s}_{hb}")
            nc.vector.reciprocal(out=rspr, in_=rsp)
            rrow = spool.tile([1, 512], F32, tag="rrow", bufs=1, name=f"rrow{hidx}_{s}_{hb}")
            nc.gpsimd.dma_start(out=rrow[0:1, :].rearrange("p (a b) -> p a b", a=128), in_=rspr)
            rec = recpool.tile([128, 512], F32, tag="rec", name=f"rec{hidx}_{s}_{hb}")
            nc.gpsimd.partition_broadcast(out_ap=rec[:, :], in_ap=rrow[0:1, :])
            out.append(rec)
        recs[(hidx, s)] = out

    def emit_norm(ps):
        p, h, s, hidx = ps["p"], ps["h"], ps["s"], ps["hidx"]
        recA, recB = recs[(hidx, s)]
        ucols = slice(h * 1024 + s * 512, h * 1024 + (s + 1) * 512)
        ut = ps["ut"]
        nc.vector.tensor_mul(uhat[p][0:64, ucols], ut[0:64, :], recA[0:64, :])
        nc.vector.tensor_mul(uhat[p][64:128, ucols], ut[64:128, :], recB[64:128, :])

    def pump_pv(cur_hidx, cur_step, budget):
        while budget > 0 and passes:
            ps = passes[0]
            if ps["kt"] >= 16:
                if recs.get((ps["hidx"], ps["s"])) is None:
                    break  # rec not emitted yet (same-half): wait for end block
                emit_norm(ps)
                passes.pop(0)
                continue
            kt = ps["kt"]
            if ps["hidx"] == cur_hidx and kt > cur_step - 2:
                break
            if not v_emitted[kt]:
                break
            if ps["ut"] is None:
                ps["ut"] = psU.tile([128, 512], F32, tag="ut", name=f"ut{ps['hidx']}_{ps['s']}")
            e = eS.pop((ps["hidx"], ps["s"], kt))
            p = ps["p"]
            st, sp = (kt == 0), (kt == 15)
            nc.tensor.matmul(ps["ut"][0:64, :], v_sb[kt][:, p * 128:p * 128 + 64], e[:, 0:512], start=st, stop=sp)
            nc.tensor.matmul(ps["ut"][64:128, :], v_sb[kt][:, p * 128 + 64:(p + 1) * 128], e[:, 512:1024], start=st, stop=sp)
            ps["kt"] += 1
            budget -= 1

    # ---- dense filler schedule -------------------------------------------
    QK = lambda f, t: (lambda: qk_chain(f, t))
    V = lambda t: (lambda: v_chain(t))
    PJ = lambda qt, es: (lambda: proj_chain(qt, es))
    fillers = [
        # p0h0: v stream + K-p0 incremental + pair-1 features early
        [V(2), QK(1, 1), V(3), QK(2, 0), V(4), QK(1, 2), V(5), QK(2, 1),
         V(6), QK(1, 3), V(7), QK(3, 0), V(8), V(9), V(10), V(11)],
        # p1h0
        [V(12), V(13), QK(3, 1), V(14), QK(3, 2), V(15), QK(3, 3), QK(4, 0),
         QK(4, 1), QK(5, 0), QK(5, 1)],
        # p2h0
        [QK(5, 2), QK(5, 3), QK(6, 0), QK(6, 1), QK(7, 0), QK(7, 1)],
        # p3h0
        [QK(7, 2), QK(7, 3), QK(0, 2), QK(0, 3), QK(2, 2), QK(2, 3)],
        # p0h1
        [QK(4, 2), QK(4, 3), QK(6, 2), QK(6, 3)],
        # p1h1 .. p3h1: proj for h0 q-tiles
        [PJ(qt, es) for qt in range(0, 3) for es in range(2)],
        [PJ(qt, es) for qt in range(3, 6) for es in range(2)],
        [PJ(qt, es) for qt in range(6, 8) for es in range(2)],
    ]

    # ---- lead-in ----------------------------------------------------------
    qk_chain(0, 0)
    qk_chain(0, 1)
    qk_chain(1, 0)
    v_chain(0)
    v_chain(1)

    # ---- main loop --------------------------------------------------------
    for hidx, (p, h) in enumerate(HALVES):
        fl = list(fillers[hidx])
        for kt in range(16):
            emit_scores_exps(hidx, p, h, kt)
            if fl:
                fl.pop(0)()
            if kt == 8:
                passes.append({"hidx": hidx, "p": p, "h": h, "s": 0, "kt": 0, "ut": None})
            pump_pv(hidx, kt, 3)
        rec_chain(hidx, 0)
        rec_chain(hidx, 1)
        passes.append({"hidx": hidx, "p": p, "h": h, "s": 1, "kt": 0, "ut": None})

    # ---- tail -------------------------------------------------------------
    guard = 0
    while passes and guard < 200:
        pump_pv(-1, 99, 4)
        guard += 1
    for qt in range(8, 16):
        for es in range(2):
            proj_chain(qt, es)


_NC_CACHE = {}


def _build_nc():
    if "nc" in _NC_CACHE:
        return _NC_CACHE["nc"]
    nc = bacc.Bacc("TRN2", target_bir_lowering=False, debug=False, num_devices=N_CORES)
    xT = nc.dram_tensor("xT", [D, NT], BF, kind="ExternalInput").ap()
    wqk = nc.dram_tensor("wqk", [D, 1024], BF, kind="ExternalInput").ap()
    wv = nc.dram_tensor("wv", [D, 512], BF, kind="ExternalInput").ap()
    wp = nc.dram_tensor("wp", [512, 1024], BF, kind="ExternalInput").ap()
    y = nc.dram_tensor("y", [NT, 1024], F32, kind="ExternalOutput").ap()
    from contextlib import ExitStack

    with tile.TileContext(nc) as tc, ExitStack() as ctx:
        _body(tc, ctx, y, xT, wqk, wv, wp)
    nc.compile()
    _NC_CACHE["nc"] = nc
    return nc


def _prepare_in_maps(x, W_qkv, W_proj):
    x = np.asarray(x, dtype=np.float32)
    W_qkv = np.asarray(W_qkv, dtype=np.float32)
    W_proj = np.asarray(W_proj, dtype=np.float32)
    in_maps = []
    for c in range(N_CORES):
        b, hg = divmod(c, 2)
        cs = slice(hg * 512, (hg + 1) * 512)
        xTc = np.ascontiguousarray(x[b].T).astype(BF16)
        Qc = W_qkv[:, 0:1024][:, cs]
        Kc = W_qkv[:, 1024:2048][:, cs]
        # per-pair interleave: [Q-pair0 | K-pair0 | Q-pair1 | K-pair1 | ...]
        blocks = []
        for p in range(4):
            blocks.append(Qc[:, p * 128:(p + 1) * 128])
            blocks.append(Kc[:, p * 128:(p + 1) * 128])
        wqk = np.ascontiguousarray(np.concatenate(blocks, axis=1)).astype(BF16)
        wv = np.ascontiguousarray(W_qkv[:, 2048:3072][:, cs]).astype(BF16)
        wp = np.ascontiguousarray(W_proj[cs, :]).astype(BF16)
        in_maps.append({"xT": xTc, "wqk": wqk, "wv": wv, "wp": wp})
    return in_maps


def _run(x, W_qkv, W_proj, b_proj, trace=False):
    nc = _build_nc()
    in_maps = _prepare_in_maps(x, W_qkv, W_proj)
    res = bass_utils.run_bass_kernel_spmd(
        nc, in_maps, core_ids=list(range(N_CORES)), trace=trace
    )
    b_proj = np.asarray(b_proj, dtype=np.float32)
    y = np.empty((4, NT, D), dtype=np.float32)
    for b in range(4):
        y[b] = res.results[2 * b]["y"] + res.results[2 * b + 1]["y"] + b_proj[None, :]
    return y, res


def kernel(x, W_qkv, W_proj, b_proj):
    y, _ = _run(x, W_qkv, W_proj, b_proj, trace=False)
    return y
